# revision 40
# baseline (speedup 1.0000x reference)
"""MultiHeadAttention TRN2 kernel: 8-way (batch x head-half) sharding, bf16.

Core c handles batch b=c//2, heads g*8..g*8+8 where g=c%2 (4 head-pairs).

Per core, all matmul inputs are bf16 (fp32 PSUM accumulation): Q^T/K^T
projections (lhsT=W-slice, rhs=X^T pre-transposed on the host), V in natural
layout with a fused ones-column (softmax denominators fall out of the P@V
matmul), scores computed transposed (keys on partition, so the key mask folds
into the per-partition ACT bias of the exp), PV matmul -> ctx^T, f32
normalization fused into the PSUM eviction, partial FC (row-slice of Wfc).
The two half-head partials per batch are summed on the host while unsharding.

Masked keys contribute exactly zero attention weight (exp of -1e30 underflows
to 0 in the reference), so the host compacts K/V inputs to the unmasked keys
(padded to a fixed LK with a -30000 bias so padding also exps to exactly 0).
If a mask ever leaves more than LK keys unmasked, kernel() falls back to a
host computation.

Head-PAIR processing: each attention unit covers one head pair x 512 queries.
The two scores matmuls have K=dk=64 and are emitted adjacently with lhsT/rhs
base partitions 0 and 64, so they land in different PE row groups
(tile_position auto-derivation) and stream concurrently through the array.
One [128,1024] exp instruction covers both heads' scores (the mask bias is
per key partition, shared by the pair). The exp pass on the Scalar engine
(~1 elem/lane/cycle) is the critical resource; K^T/Q^T/FC matmuls are
interleaved into the attention stream so PE work hides under the ACT-bound
phase.
"""

import numpy as np
import ml_dtypes

import concourse.mybir as mybir
import concourse.tile as tile
from concourse import bacc
from concourse.bass import ts

F32 = mybir.dt.float32
F32R = mybir.dt.float32r
BF16 = mybir.dt.bfloat16
AF = mybir.ActivationFunctionType
NPBF = ml_dtypes.bfloat16

BS, L, D = 4, 2048, 1024
NCORES = 8
H = 8                 # heads per core
DK = 64
HD = H * DK           # 512: head dims per core
LK = 1152             # padded compacted-key length (9 chunks of 128)
NEGB = -30000.0       # masked/padded-key bias (exp underflows to exactly 0)
SCALE = 1.0 / 8.0     # 1/sqrt(DK)


def _build(lk):
    kck = lk // 128           # key chunks
    nc = bacc.Bacc()
    # chunk-major layouts so each tensor lands in one (or few) big DMAs:
    # element [p, k, j] = M[k*128+p, j] for the 1024-row operands.
    xt = nc.declare_dram_parameter("xt", [4, 128, 8, 512], BF16, isOutput=False)
    xkv = nc.declare_dram_parameter("xkv", [128, 8, lk], BF16, isOutput=False)
    wq = nc.declare_dram_parameter("wq", [4, 128, 8, 128], BF16, isOutput=False)
    wk = nc.declare_dram_parameter("wk", [4, 128, 8, 128], BF16, isOutput=False)
    wv = nc.declare_dram_parameter("wv", [128, 8, HD], BF16, isOutput=False)
    wfc = nc.declare_dram_parameter("wfc", [128, 4, D], BF16, isOutput=False)
    # bias columns: 0-3 = bk tiles, 4-7 = bq tiles, 8-15 = bfc*0.5 tiles
    bias = nc.declare_dram_parameter("bias", [128, 16], F32, isOutput=False)
    bvr = nc.declare_dram_parameter("bvr", [1, HD], BF16, isOutput=False)
    mb = nc.declare_dram_parameter("mb", [128, kck], F32, isOutput=False)
    outp = nc.declare_dram_parameter("out", [8, 128, L], BF16, isOutput=True)
    import os as _os
    _dbg = bool(_os.environ.get("KDBG"))
    if _dbg:
        ctxd = nc.declare_dram_parameter("ctxd", [4, 128, L], BF16,
                                         isOutput=True)
        qtd = nc.declare_dram_parameter("qtd", [4, 128, L], BF16,
                                        isOutput=True)
        ktd = nc.declare_dram_parameter("ktd", [4, 128, lk], BF16,
                                        isOutput=True)
        dend = nc.declare_dram_parameter("dend", [2, 512], F32,
                                         isOutput=True)
        rbsd = nc.declare_dram_parameter("rbsd", [2, 64, 512], F32,
                                         isOutput=True)

    with tile.TileContext(nc) as tc:
        with tc.tile_pool(name="const", bufs=1) as pc, \
             tc.tile_pool(name="xs", bufs=1) as p_x, \
             tc.tile_pool(name="wres", bufs=1) as p_w, \
             tc.tile_pool(name="wfcp", bufs=1) as p_wfc, \
             tc.tile_pool(name="bias", bufs=1) as p_b, \
             tc.tile_pool(name="qt", bufs=4) as p_qt, \
             tc.tile_pool(name="kt", bufs=4) as p_kt, \
             tc.tile_pool(name="v", bufs=9) as p_v, \
             tc.tile_pool(name="ctx", bufs=4) as p_ctx, \
             tc.tile_pool(name="pt", bufs=4, side="right") as p_pt, \
             tc.tile_pool(name="smallB", bufs=3, side="right") as p_sm, \
             tc.tile_pool(name="ev", bufs=4, side="right") as p_ev, \
             tc.tile_pool(name="ps", bufs=2, space="PSUM") as PS:
            # warm the ACT exp table before any real dependency exists
            dmy = pc.tile([1, 8], F32)
            nc.vector.memset(dmy[:], 0.0)
            dmy2 = pc.tile([1, 8], F32)
            nc.scalar.activation(dmy2[:], dmy[:], AF.Exp)

            # constants (memset is fp32-only; convert via tensor_copy)
            ones_f = pc.tile([1, 128], F32)
            nc.vector.memset(ones_f[:], 1.0)
            ones_b = pc.tile([1, 128], BF16)
            nc.vector.tensor_copy(ones_b[:], ones_f[:])
            onesv_f = pc.tile([128, 8, 1], F32)
            nc.vector.memset(onesv_f[:], 1.0)
            onesv = pc.tile([128, 8, 1], BF16)
            nc.vector.tensor_copy(onesv[:], onesv_f[:])
            mb_sb = pc.tile([128, kck], F32)
            nc.sync.dma_start(out=mb_sb[:], in_=mb[:])
            bv_sb = pc.tile([1, HD], BF16)
            nc.sync.dma_start(out=bv_sb[:], in_=bvr[:])

            # weights + inputs in a handful of large DMAs, ordered by
            # consumption so the lead-in (kt pair 0 + V chunk 0 + Q block 0)
            # unblocks as early as possible.
            # pair-major weight slices + column-block xkv/xt DMAs, ordered so
            # the first attention unit's data (pair 0, key block 0, query
            # block 0) lands first.
            wk_p = [p_w.tile([128, 8, 128], BF16, tag=f"wkp{t}",
                             name=f"wkp{t}") for t in range(4)]
            wq_p = [p_w.tile([128, 8, 128], BF16, tag=f"wqp{t}",
                             name=f"wqp{t}") for t in range(4)]
            xt_b = [p_x.tile([128, 8, 512], BF16, tag=f"xtb{n}",
                             name=f"xtb{n}") for n in range(4)]
            xkv_s = p_x.tile([128, 8, lk], BF16, tag="xkv")
            bias_s = p_b.tile([128, 16], F32, tag="bias")
            wv_s = p_w.tile([128, 8, HD], BF16, tag="wv")
            wfc_s = p_wfc.tile([128, 4, D], BF16, tag="wfc")

            nc.sync.dma_start(out=wk_p[0][:], in_=wk[0])
            nc.sync.dma_start(out=bias_s[:], in_=bias[:])
            nc.sync.dma_start(out=xkv_s[:, :, 0:512], in_=xkv[:, :, 0:512])
            nc.sync.dma_start(out=wq_p[0][:], in_=wq[0])
            nc.sync.dma_start(out=xt_b[0][:], in_=xt[0])
            nc.sync.dma_start(out=wv_s[:], in_=wv[:])
            nc.sync.dma_start(out=xkv_s[:, :, 512:1024],
                              in_=xkv[:, :, 512:1024])
            nc.sync.dma_start(out=xkv_s[:, :, 1024:lk],
                              in_=xkv[:, :, 1024:lk])
            for t in range(1, 4):
                nc.sync.dma_start(out=wk_p[t][:], in_=wk[t])
                nc.sync.dma_start(out=wq_p[t][:], in_=wq[t])
            for n in range(1, 4):
                nc.sync.dma_start(out=xt_b[n][:], in_=xt[n])
            nc.sync.dma_start(out=wfc_s[:], in_=wfc[:])

            kt_t = [p_kt.tile([128, lk], BF16, tag="kt", name=f"kt{i}")
                    for i in range(4)]
            v_t = [p_v.tile([128, 8, 65], BF16, tag="v", name=f"v{i}")
                   for i in range(kck)]
            qt_t = [p_qt.tile([128, L], BF16, tag="qt", name=f"qt{i}")
                    for i in range(4)]
            ctx_t = [p_ctx.tile([128, L], BF16, tag="ctx", name=f"ctx{i}")
                     for i in range(4)]

            nkv = (lk + 511) // 512

            def kt_block(t, n):
                """K^T tile t (head pair t), key columns [n*512, ...)."""
                c0 = n * 512
                w = min(512, lk - c0)
                ps = PS.tile([128, 512], F32, tag="mm", name="psk")
                for k in range(8):
                    nc.tensor.matmul(ps[:, :w], wk_p[t][:, k, :],
                                     xkv_s[:, k, c0:c0 + w],
                                     start=(k == 0), stop=(k == 7))
                nc.vector.tensor_scalar_add(
                    kt_t[t][:, c0:c0 + w], ps[:, :w], bias_s[:, t:t + 1])

            def v_chunk(m):
                """V rows for key chunk m, all 8 heads + ones column."""
                ps = PS.tile([128, 512], F32, tag="mm", name="psv")
                for k in range(8):
                    nc.tensor.matmul(ps[:], xkv_s[:, k, ts(m, 128)],
                                     wv_s[:, k, :],
                                     start=(k == 0), stop=False)
                nc.tensor.matmul(ps[:], ones_b[:, :128], bv_sb[:],
                                 start=False, stop=True)
                nc.vector.tensor_copy(
                    v_t[m][:, :, 0:64],
                    ps[:].rearrange("p (h d) -> p h d", h=8))
                nc.vector.tensor_copy(v_t[m][:, :, 64:65], onesv[:])

            def qt_block(t, n):
                """Q^T tile t, query columns [n*512, (n+1)*512)."""
                ps = PS.tile([128, 512], F32, tag="mm", name="psq")
                for k in range(8):
                    nc.tensor.matmul(ps[:], wq_p[t][:, k, :],
                                     xt_b[n][:, k, :],
                                     start=(k == 0), stop=(k == 7))
                nc.vector.tensor_scalar_add(qt_t[t][:, ts(n, 512)],
                                            ps[:], bias_s[:, 4 + t:5 + t])

            def attn_unit(p, qn):
                """Head pair p (heads 2p, 2p+1), queries [qn*512, qn*512+512).

                Emits prerequisites first, then pops one filler item per key
                chunk so projection/FC PE work interleaves into the ACT-bound
                stream.
                """
                q0 = qn * 512
                hA, hB = 2 * p, 2 * p + 1
                cA = PS.tile([65, 512], F32, tag="ctxp", name="cA")
                cB = PS.tile([65, 512], F32, tag="ctxp", name="cB")
                ensure(("kt", p, 0))
                ensure(("kt", p, 1))
                ensure(("kt", p, 2))
                ensure(("qt", p, qn))
                for kc in range(kck):
                    ensure(("v", kc))
                    sps = PS.tile([128, 1024], F32, tag="s", name="s")
                    # two K=64 matmuls in different PE row groups (base
                    # partitions 0/64) -> concurrent array streams
                    nc.tensor.matmul(sps[:, 0:512],
                                     kt_t[p][0:64, ts(kc, 128)],
                                     qt_t[p][0:64, q0:q0 + 512],
                                     start=True, stop=True)
                    nc.tensor.matmul(sps[:, 512:1024],
                                     kt_t[p][64:128, ts(kc, 128)],
                                     qt_t[p][64:128, q0:q0 + 512],
                                     start=True, stop=True)
                    pt = p_pt.tile([128, 1024], BF16, tag="pt", name="pt")
                    nc.scalar.activation(pt[:], sps[:], AF.Exp,
                                         bias=mb_sb[:, kc:kc + 1], scale=SCALE)
                    st, sp = (kc == 0), (kc == kck - 1)
                    nc.tensor.matmul(cA[:], v_t[kc][:, hA, :],
                                     pt[:, 0:512], start=st, stop=sp)
                    nc.tensor.matmul(cB[:], v_t[kc][:, hB, :],
                                     pt[:, 512:1024], start=st, stop=sp)
                    pop_filler()
                for hi, (cp, row0) in enumerate(((cA, 0), (cB, 64))):
                    rb1 = p_sm.tile([1, 512], F32, tag="rb1", name="rb1")
                    nc.vector.reciprocal(rb1[:], cp[64:65, :])
                    rbs = p_sm.tile([64, 512], F32, tag="rbs", name="rbs")
                    nc.gpsimd.partition_broadcast(rbs[:], rb1[:], channels=64)
                    if _dbg and (p, qn) == (1, 0):
                        nc.sync.dma_start(out=dend[hi:hi + 1], in_=rb1[:])
                        nc.sync.dma_start(out=rbsd[hi], in_=rbs[:])
                    nc.vector.tensor_mul(
                        ctx_t[p][row0:row0 + 64, q0:q0 + 512],
                        cp[0:64, :], rbs[:])

            def fc_mtile(n, m):
                """FC output rows [m*128, ...) over query cols [n*512, ...)."""
                ps = PS.tile([128, 512], F32, tag="mm", name="f")
                for k in range(4):
                    nc.tensor.matmul(ps[:], wfc_s[:, k, ts(m, 128)],
                                     ctx_t[k][:, ts(n, 512)],
                                     start=(k == 0), stop=(k == 3))
                ev = p_ev.tile([128, 512], BF16, tag="ev")
                nc.vector.tensor_scalar_add(ev[:], ps[:], bias_s[:, 8 + m:9 + m])
                nc.sync.dma_start(out=outp[m][:, ts(n, 512)], in_=ev[:])

            # ---- emission: minimal lead-in, then the ACT-bound attention
            # ---- stream. All other projection/FC PE work is registered in a
            # ---- work map; units pop one item per key chunk to fill PE
            # ---- slack, and ensure() emits any not-yet-emitted prerequisite
            # ---- BEFORE the instructions that read it (the Tile dependency
            # ---- tracker follows emission order, so a reader emitted before
            # ---- its writer would read garbage).
            from collections import deque

            work = {}
            order = deque()

            def add(key, fn):
                work[key] = fn
                order.append(key)

            def ensure(key):
                fn = work.pop(key, None)
                if fn is not None:
                    fn()

            def pop_filler():
                while order:
                    k = order.popleft()
                    fn = work.pop(k, None)
                    if fn is not None:
                        fn()
                        return

            kt_block(0, 0)
            kt_block(0, 1)
            kt_block(0, 2)
            v_chunk(0)
            qt_block(0, 0)

            for m in range(1, kck):
                add(("v", m), lambda m=m: v_chunk(m))
            for t in range(1, 4):
                for n in range(3):
                    add(("kt", t, n), lambda t=t, n=n: kt_block(t, n))
                add(("qt", t, 0), lambda t=t: qt_block(t, 0))

            for qn in range(4):
                for p in range(4):
                    attn_unit(p, qn)
                    if qn < 3:
                        add(("qt", p, qn + 1),
                            lambda p=p, qn=qn: qt_block(p, qn + 1))
                for m in range(8):
                    add(("fc", qn, m), lambda qn=qn, m=m: fc_mtile(qn, m))
            while order:
                pop_filler()
            if _dbg:
                for t in range(4):
                    nc.sync.dma_start(out=ctxd[t], in_=ctx_t[t][:])
                    nc.sync.dma_start(out=qtd[t], in_=qt_t[t][:])
                    nc.sync.dma_start(out=ktd[t], in_=kt_t[t][:])

    nc.finalize()
    return nc


class _Runner:
    """Compile-once wrapper around the run_bass_via_pjrt shard_map path."""

    def __init__(self, nc):
        import jax
        from jax.sharding import Mesh, PartitionSpec

        from concourse import bass2jax, mybir as mb

        try:
            from jax.experimental.shard_map import shard_map
        except ImportError:
            from jax.shard_map import shard_map

        bass2jax.install_neuronx_cc_hook()
        self._nc = nc
        partition_name = (nc.partition_id_tensor.name
                          if nc.partition_id_tensor else None)
        in_names, out_names, out_avals = [], [], []
        self._zero_shapes = []
        for alloc in nc.m.functions[0].allocations:
            if not isinstance(alloc, mb.MemoryLocationSet):
                continue
            name = alloc.memorylocations[0].name
            if alloc.kind == "ExternalInput":
                if name != partition_name:
                    in_names.append(name)
            elif alloc.kind == "ExternalOutput":
                out_names.append(name)
                shape = tuple(alloc.tensor_shape)
                dtype = mb.dt.np(alloc.dtype)
                out_avals.append(jax.core.ShapedArray(shape, dtype))
                self._zero_shapes.append((shape, dtype))
        self._n_params = len(in_names)
        n_outs = len(out_avals)
        self._in_names = list(in_names)
        self._out_names = list(out_names)
        self._out_avals = out_avals
        all_in = in_names + out_names
        if partition_name is not None:
            all_in.append(partition_name)

        def _body(*args):
            operands = list(args)
            if partition_name is not None:
                operands.append(bass2jax.partition_id_tensor())
            return tuple(bass2jax._bass_exec_p.bind(
                *operands,
                out_avals=tuple(out_avals),
                in_names=tuple(all_in),
                out_names=tuple(out_names),
                lowering_input_output_aliases=(),
                sim_require_finite=True,
                sim_require_nnan=True,
                nc=nc,
            ))

        devices = jax.devices()[:NCORES]
        mesh = Mesh(np.asarray(devices), ("core",))
        self.mesh = mesh
        nin = self._n_params + n_outs
        self._sharded = jax.jit(
            shard_map(_body, mesh=mesh,
                      in_specs=(PartitionSpec("core"),) * nin,
                      out_specs=(PartitionSpec("core"),) * n_outs,
                      check_rep=False),
            donate_argnums=tuple(range(self._n_params, nin)),
            keep_unused=True,
        )

    def run(self, in_maps):
        import jax
        concat_in = [
            np.concatenate([np.asarray(in_maps[c][name])
                            for c in range(NCORES)], axis=0)
            for name in self._in_names
        ]
        concat_zeros = [np.zeros((NCORES * s[0], *s[1:]), d)
                        for s, d in self._zero_shapes]
        out_arrs = self._sharded(*concat_in, *concat_zeros)
        jax.block_until_ready(out_arrs)
        return [
            {name: np.asarray(out_arrs[i]).reshape(
                NCORES, *self._out_avals[i].shape)[c]
             for i, name in enumerate(self._out_names)}
            for c in range(NCORES)
        ]


_RUNNERS = {}


def _get_runner(lk):
    if lk not in _RUNNERS:
        _RUNNERS[lk] = _Runner(_build(lk))
    return _RUNNERS[lk]


def _prep_in_maps(x, mask, Wq, bq, Wk, bk, Wv, bv, Wfc, bfc):
    """Shard + lay out the full inputs for the 8 cores.

    Returns (in_maps, lk) or (None, None) if the mask leaves more than LK
    keys unmasked in some batch (host fallback).
    """
    keep = [np.nonzero(mask[b] == 0)[0] for b in range(BS)]
    if max(len(kp) for kp in keep) > LK or min(len(kp) for kp in keep) == 0:
        # too many unmasked keys for the compiled shape, or a fully-masked
        # batch (reference degenerates to uniform attention there)
        return None, None
    lk = LK

    in_maps = []
    for c in range(NCORES):
        b, g = c // 2, c % 2
        sl = slice(g * HD, (g + 1) * HD)
        kp = keep[b]
        xkv_b = np.zeros((lk, D), np.float32)
        xkv_b[:len(kp)] = x[b][kp]
        biask = np.where(np.arange(lk) < len(kp), 0.0, NEGB).astype(np.float32)
        bias_cols = np.concatenate([
            bk[sl].reshape(4, 128).T,
            bq[sl].reshape(4, 128).T,
            (bfc * 0.5).reshape(8, 128).T,
        ], axis=1).astype(np.float32)
        in_maps.append({
            "xt": np.ascontiguousarray(
                x[b].T.reshape(8, 128, 4, 512).transpose(2, 1, 0, 3)
            ).astype(NPBF),
            "xkv": np.ascontiguousarray(
                xkv_b.T.reshape(8, 128, lk).transpose(1, 0, 2)).astype(NPBF),
            "wq": np.ascontiguousarray(
                Wq[:, sl].reshape(8, 128, 4, 128).transpose(2, 1, 0, 3)
            ).astype(NPBF),
            "wk": np.ascontiguousarray(
                Wk[:, sl].reshape(8, 128, 4, 128).transpose(2, 1, 0, 3)
            ).astype(NPBF),
            "wv": np.ascontiguousarray(
                Wv[:, sl].reshape(8, 128, HD).transpose(1, 0, 2)).astype(NPBF),
            "wfc": np.ascontiguousarray(
                Wfc[sl, :].reshape(4, 128, D).transpose(1, 0, 2)).astype(NPBF),
            "bias": np.ascontiguousarray(bias_cols),
            "bvr": np.ascontiguousarray(bv[sl]).reshape(1, HD).astype(NPBF),
            "mb": np.ascontiguousarray(biask.reshape(lk // 128, 128).T),
        })
    return in_maps, lk


def _host_reference(x, mask, Wq, bq, Wk, bk, Wv, bv, Wfc, bfc):
    """Numpy fallback, bit-compatible with the reference semantics."""
    out = np.empty((BS, L, D), np.float32)
    for b in range(BS):
        q = (x[b] @ Wq + bq).reshape(L, 16, DK).transpose(1, 0, 2)
        k = (x[b] @ Wk + bk).reshape(L, 16, DK).transpose(1, 0, 2)
        v = (x[b] @ Wv + bv).reshape(L, 16, DK).transpose(1, 0, 2)
        s = np.einsum("hqd,hkd->hqk", q, k) * SCALE
        m = mask[b].astype(np.float32)[None, None, :]
        s = s * (1.0 - m) + m * (-1e30)
        s = s - s.max(axis=-1, keepdims=True)
        p = np.exp(s)
        p /= p.sum(axis=-1, keepdims=True)
        o = np.einsum("hqk,hkd->hqd", p, v).transpose(1, 0, 2).reshape(L, D)
        out[b] = o @ Wfc + bfc
    return out


def kernel(x, mask, Wq, bq, Wk, bk, Wv, bv, Wfc, bfc, **_unused):
    x = np.asarray(x, np.float32)
    mask = np.asarray(mask)
    Wq, bq = np.asarray(Wq, np.float32), np.asarray(bq, np.float32)
    Wk, bk = np.asarray(Wk, np.float32), np.asarray(bk, np.float32)
    Wv, bv = np.asarray(Wv, np.float32), np.asarray(bv, np.float32)
    Wfc, bfc = np.asarray(Wfc, np.float32), np.asarray(bfc, np.float32)

    in_maps, lk = _prep_in_maps(x, mask, Wq, bq, Wk, bk, Wv, bv, Wfc, bfc)
    if in_maps is None:
        return _host_reference(x, mask, Wq, bq, Wk, bk, Wv, bv, Wfc, bfc)
    results = _get_runner(lk).run(in_maps)

    out = np.empty((BS, L, D), np.float32)
    for b in range(BS):
        p0 = results[2 * b]["out"].reshape(D, L).astype(np.float32)
        p1 = results[2 * b + 1]["out"].reshape(D, L).astype(np.float32)
        out[b] = (p0 + p1).T
    return out


# revision 44
# speedup vs baseline: 3.8006x; 3.8006x over previous
"""MultiHeadAttention TRN2 kernel: 8-way (batch x head-half) sharding, bf16.

Core c handles batch b=c//2, heads g*8..g*8+8 where g=c%2 (4 head-pairs).

Per core, all matmul inputs are bf16 (fp32 PSUM accumulation): Q^T/K^T
projections (lhsT=W-slice, rhs=X^T pre-transposed on the host), V in natural
layout with a fused ones-column (softmax denominators fall out of the P@V
matmul), scores computed transposed (keys on partition, so the key mask folds
into the per-partition ACT bias of the exp), PV matmul -> ctx^T, f32
normalization fused into the PSUM eviction, partial FC (row-slice of Wfc).
The two half-head partials per batch are summed on the host while unsharding.

Masked keys contribute exactly zero attention weight (exp of -1e30 underflows
to 0 in the reference), so the host compacts K/V inputs to the unmasked keys
(padded to a fixed LK with a -30000 bias so padding also exps to exactly 0).
If a mask ever leaves more than LK keys unmasked, kernel() falls back to a
host computation.

Head-PAIR processing: each attention unit covers one head pair x 512 queries.
The two scores matmuls have K=dk=64 and are emitted adjacently with lhsT/rhs
base partitions 0 and 64, so they land in different PE row groups
(tile_position auto-derivation) and stream concurrently through the array.
One [128,1024] exp instruction covers both heads' scores (the mask bias is
per key partition, shared by the pair). The exp pass on the Scalar engine
(~1 elem/lane/cycle) is the critical resource; K^T/Q^T/FC matmuls are
interleaved into the attention stream so PE work hides under the ACT-bound
phase.
"""

import numpy as np
import ml_dtypes

import concourse.mybir as mybir
import concourse.tile as tile
from concourse import bacc
from concourse.bass import ts

F32 = mybir.dt.float32
F32R = mybir.dt.float32r
BF16 = mybir.dt.bfloat16
AF = mybir.ActivationFunctionType
NPBF = ml_dtypes.bfloat16

BS, L, D = 4, 2048, 1024
NCORES = 8
H = 8                 # heads per core
DK = 64
HD = H * DK           # 512: head dims per core
LK = 1152             # padded compacted-key length (9 chunks of 128)
NEGB = -30000.0       # masked/padded-key bias (exp underflows to exactly 0)
SCALE = 1.0 / 8.0     # 1/sqrt(DK)


def _build(lk, reps=1):
    kck = lk // 128           # key chunks
    nc = bacc.Bacc()
    # chunk-major layouts so each tensor lands in one (or few) big DMAs:
    # element [p, k, j] = M[k*128+p, j] for the 1024-row operands.
    xt = nc.declare_dram_parameter("xt", [4, 128, 8, 512], BF16, isOutput=False)
    xkv = nc.declare_dram_parameter("xkv", [128, 8, lk], BF16, isOutput=False)
    wq = nc.declare_dram_parameter("wq", [4, 128, 8, 128], BF16, isOutput=False)
    wk = nc.declare_dram_parameter("wk", [4, 128, 8, 128], BF16, isOutput=False)
    wv = nc.declare_dram_parameter("wv", [128, 8, HD], BF16, isOutput=False)
    wfc = nc.declare_dram_parameter("wfc", [128, 4, D], BF16, isOutput=False)
    # bias columns: 0-3 = bk tiles, 4-7 = bq tiles, 8-15 = bfc*0.5 tiles
    bias = nc.declare_dram_parameter("bias", [128, 16], F32, isOutput=False)
    bvr = nc.declare_dram_parameter("bvr", [1, HD], BF16, isOutput=False)
    mb = nc.declare_dram_parameter("mb", [128, kck], F32, isOutput=False)
    outp = nc.declare_dram_parameter("out", [8, 128, L], BF16, isOutput=True)
    import os as _os
    _dbg = bool(_os.environ.get("KDBG"))
    if _dbg:
        ctxd = nc.declare_dram_parameter("ctxd", [4, 128, L], BF16,
                                         isOutput=True)
        qtd = nc.declare_dram_parameter("qtd", [4, 128, L], BF16,
                                        isOutput=True)
        ktd = nc.declare_dram_parameter("ktd", [4, 128, lk], BF16,
                                        isOutput=True)
        dend = nc.declare_dram_parameter("dend", [2, 512], F32,
                                         isOutput=True)
        rbsd = nc.declare_dram_parameter("rbsd", [2, 64, 512], F32,
                                         isOutput=True)

    with tile.TileContext(nc) as tc:
        with tc.tile_pool(name="const", bufs=1) as pc, \
             tc.tile_pool(name="xs", bufs=1) as p_x, \
             tc.tile_pool(name="wres", bufs=1) as p_w, \
             tc.tile_pool(name="wfcp", bufs=1) as p_wfc, \
             tc.tile_pool(name="bias", bufs=1) as p_b, \
             tc.tile_pool(name="qt", bufs=4) as p_qt, \
             tc.tile_pool(name="kt", bufs=4) as p_kt, \
             tc.tile_pool(name="v", bufs=9) as p_v, \
             tc.tile_pool(name="ctx", bufs=4) as p_ctx, \
             tc.tile_pool(name="pt", bufs=4, side="right") as p_pt, \
             tc.tile_pool(name="smallB", bufs=3, side="right") as p_sm, \
             tc.tile_pool(name="ev", bufs=4, side="right") as p_ev, \
             tc.tile_pool(name="ps", bufs=2, space="PSUM") as PS:
            # warm the ACT exp table before any real dependency exists
            dmy = pc.tile([1, 8], F32)
            nc.vector.memset(dmy[:], 0.0)
            dmy2 = pc.tile([1, 8], F32)
            nc.scalar.activation(dmy2[:], dmy[:], AF.Exp)

            # constants (memset is fp32-only; convert via tensor_copy)
            ones_f = pc.tile([1, 128], F32)
            nc.vector.memset(ones_f[:], 1.0)
            ones_b = pc.tile([1, 128], BF16)
            nc.vector.tensor_copy(ones_b[:], ones_f[:])
            onesv_f = pc.tile([128, 8, 1], F32)
            nc.vector.memset(onesv_f[:], 1.0)
            onesv = pc.tile([128, 8, 1], BF16)
            nc.vector.tensor_copy(onesv[:], onesv_f[:])
            mb_sb = pc.tile([128, kck], F32)
            nc.sync.dma_start(out=mb_sb[:], in_=mb[:])
            bv_sb = pc.tile([1, HD], BF16)
            nc.sync.dma_start(out=bv_sb[:], in_=bvr[:])

            # weights + inputs in a handful of large DMAs, ordered by
            # consumption so the lead-in (kt pair 0 + V chunk 0 + Q block 0)
            # unblocks as early as possible.
            # pair-major weight slices + column-block xkv/xt DMAs, ordered so
            # the first attention unit's data (pair 0, key block 0, query
            # block 0) lands first.
            wk_p = [p_w.tile([128, 8, 128], BF16, tag=f"wkp{t}",
                             name=f"wkp{t}") for t in range(4)]
            wq_p = [p_w.tile([128, 8, 128], BF16, tag=f"wqp{t}",
                             name=f"wqp{t}") for t in range(4)]
            xt_b = [p_x.tile([128, 8, 512], BF16, tag=f"xtb{n}",
                             name=f"xtb{n}") for n in range(4)]
            xkv_s = p_x.tile([128, 8, lk], BF16, tag="xkv")
            bias_s = p_b.tile([128, 16], F32, tag="bias")
            wv_s = p_w.tile([128, 8, HD], BF16, tag="wv")
            wfc_s = p_wfc.tile([128, 4, D], BF16, tag="wfc")

            def emit_input_dmas():
                nc.sync.dma_start(out=wk_p[0][:], in_=wk[0])
                nc.sync.dma_start(out=bias_s[:], in_=bias[:])
                nc.sync.dma_start(out=xkv_s[:, :, 0:512],
                                  in_=xkv[:, :, 0:512])
                nc.sync.dma_start(out=wq_p[0][:], in_=wq[0])
                nc.sync.dma_start(out=xt_b[0][:], in_=xt[0])
                nc.sync.dma_start(out=wv_s[:], in_=wv[:])
                nc.sync.dma_start(out=xkv_s[:, :, 512:1024],
                                  in_=xkv[:, :, 512:1024])
                nc.sync.dma_start(out=xkv_s[:, :, 1024:lk],
                                  in_=xkv[:, :, 1024:lk])
                for t in range(1, 4):
                    nc.sync.dma_start(out=wk_p[t][:], in_=wk[t])
                    nc.sync.dma_start(out=wq_p[t][:], in_=wq[t])
                for n in range(1, 4):
                    nc.sync.dma_start(out=xt_b[n][:], in_=xt[n])
                nc.sync.dma_start(out=wfc_s[:], in_=wfc[:])

            kt_t = [p_kt.tile([128, lk], BF16, tag="kt", name=f"kt{i}")
                    for i in range(4)]
            v_t = [p_v.tile([128, 8, 65], BF16, tag="v", name=f"v{i}")
                   for i in range(kck)]
            qt_t = [p_qt.tile([128, L], BF16, tag="qt", name=f"qt{i}")
                    for i in range(4)]
            ctx_t = [p_ctx.tile([128, L], BF16, tag="ctx", name=f"ctx{i}")
                     for i in range(4)]

            nkv = (lk + 511) // 512

            def kt_block(t, n):
                """K^T tile t (head pair t), key columns [n*512, ...)."""
                c0 = n * 512
                w = min(512, lk - c0)
                ps = PS.tile([128, 512], F32, tag="mm", name="psk")
                for k in range(8):
                    nc.tensor.matmul(ps[:, :w], wk_p[t][:, k, :],
                                     xkv_s[:, k, c0:c0 + w],
                                     start=(k == 0), stop=(k == 7))
                nc.vector.tensor_scalar_add(
                    kt_t[t][:, c0:c0 + w], ps[:, :w], bias_s[:, t:t + 1])

            def v_chunk(m):
                """V rows for key chunk m, all 8 heads + ones column."""
                ps = PS.tile([128, 512], F32, tag="mm", name="psv")
                for k in range(8):
                    nc.tensor.matmul(ps[:], xkv_s[:, k, ts(m, 128)],
                                     wv_s[:, k, :],
                                     start=(k == 0), stop=False)
                nc.tensor.matmul(ps[:], ones_b[:, :128], bv_sb[:],
                                 start=False, stop=True)
                nc.vector.tensor_copy(
                    v_t[m][:, :, 0:64],
                    ps[:].rearrange("p (h d) -> p h d", h=8))
                nc.vector.tensor_copy(v_t[m][:, :, 64:65], onesv[:])

            def qt_block(t, n):
                """Q^T tile t, query columns [n*512, (n+1)*512)."""
                ps = PS.tile([128, 512], F32, tag="mm", name="psq")
                for k in range(8):
                    nc.tensor.matmul(ps[:], wq_p[t][:, k, :],
                                     xt_b[n][:, k, :],
                                     start=(k == 0), stop=(k == 7))
                nc.vector.tensor_scalar_add(qt_t[t][:, ts(n, 512)],
                                            ps[:], bias_s[:, 4 + t:5 + t])

            def attn_unit(p, qn):
                """Head pair p (heads 2p, 2p+1), queries [qn*512, qn*512+512).

                Emits prerequisites first, then pops one filler item per key
                chunk so projection/FC PE work interleaves into the ACT-bound
                stream.
                """
                q0 = qn * 512
                hA, hB = 2 * p, 2 * p + 1
                cA = PS.tile([65, 512], F32, tag="ctxp", name="cA")
                cB = PS.tile([65, 512], F32, tag="ctxp", name="cB")
                ensure(("kt", p, 0))
                ensure(("kt", p, 1))
                ensure(("kt", p, 2))
                ensure(("qt", p, qn))
                for kc in range(kck):
                    ensure(("v", kc))
                    sps = PS.tile([128, 1024], F32, tag="s", name="s")
                    # two K=64 matmuls in different PE row groups (base
                    # partitions 0/64) -> concurrent array streams
                    nc.tensor.matmul(sps[:, 0:512],
                                     kt_t[p][0:64, ts(kc, 128)],
                                     qt_t[p][0:64, q0:q0 + 512],
                                     start=True, stop=True)
                    nc.tensor.matmul(sps[:, 512:1024],
                                     kt_t[p][64:128, ts(kc, 128)],
                                     qt_t[p][64:128, q0:q0 + 512],
                                     start=True, stop=True)
                    pt = p_pt.tile([128, 1024], BF16, tag="pt", name="pt")
                    nc.scalar.activation(pt[:], sps[:], AF.Exp,
                                         bias=mb_sb[:, kc:kc + 1], scale=SCALE)
                    st, sp = (kc == 0), (kc == kck - 1)
                    nc.tensor.matmul(cA[:], v_t[kc][:, hA, :],
                                     pt[:, 0:512], start=st, stop=sp)
                    nc.tensor.matmul(cB[:], v_t[kc][:, hB, :],
                                     pt[:, 512:1024], start=st, stop=sp)
                    pop_filler()
                for hi, (cp, row0) in enumerate(((cA, 0), (cB, 64))):
                    rb1 = p_sm.tile([1, 512], F32, tag="rb1", name="rb1")
                    nc.vector.reciprocal(rb1[:], cp[64:65, :])
                    rbs = p_sm.tile([64, 512], F32, tag="rbs", name="rbs")
                    nc.gpsimd.partition_broadcast(rbs[:], rb1[:], channels=64)
                    if _dbg and (p, qn) == (1, 0):
                        nc.sync.dma_start(out=dend[hi:hi + 1], in_=rb1[:])
                        nc.sync.dma_start(out=rbsd[hi], in_=rbs[:])
                    nc.vector.tensor_mul(
                        ctx_t[p][row0:row0 + 64, q0:q0 + 512],
                        cp[0:64, :], rbs[:])

            def fc_mtile(n, m):
                """FC output rows [m*128, ...) over query cols [n*512, ...)."""
                ps = PS.tile([128, 512], F32, tag="mm", name="f")
                for k in range(4):
                    nc.tensor.matmul(ps[:], wfc_s[:, k, ts(m, 128)],
                                     ctx_t[k][:, ts(n, 512)],
                                     start=(k == 0), stop=(k == 3))
                ev = p_ev.tile([128, 512], BF16, tag="ev")
                nc.vector.tensor_scalar_add(ev[:], ps[:], bias_s[:, 8 + m:9 + m])
                nc.sync.dma_start(out=outp[m][:, ts(n, 512)], in_=ev[:])

            # ---- emission: minimal lead-in, then the ACT-bound attention
            # ---- stream. All other projection/FC PE work is registered in a
            # ---- work map; units pop one item per key chunk to fill PE
            # ---- slack, and ensure() emits any not-yet-emitted prerequisite
            # ---- BEFORE the instructions that read it (the Tile dependency
            # ---- tracker follows emission order, so a reader emitted before
            # ---- its writer would read garbage).
            from collections import deque

            work = {}
            order = deque()

            def add(key, fn):
                work[key] = fn
                order.append(key)

            def ensure(key):
                fn = work.pop(key, None)
                if fn is not None:
                    fn()

            def pop_filler():
                while order:
                    k = order.popleft()
                    fn = work.pop(k, None)
                    if fn is not None:
                        fn()
                        return

            for _rep in range(reps):
                emit_input_dmas()
                kt_block(0, 0)
                kt_block(0, 1)
                kt_block(0, 2)
                v_chunk(0)
                qt_block(0, 0)

                for m in range(1, kck):
                    add(("v", m), lambda m=m: v_chunk(m))
                for t in range(1, 4):
                    for n in range(3):
                        add(("kt", t, n), lambda t=t, n=n: kt_block(t, n))
                    add(("qt", t, 0), lambda t=t: qt_block(t, 0))

                for qn in range(4):
                    for p in range(4):
                        attn_unit(p, qn)
                        if qn < 3:
                            add(("qt", p, qn + 1),
                                lambda p=p, qn=qn: qt_block(p, qn + 1))
                    for m in range(8):
                        add(("fc", qn, m),
                            lambda qn=qn, m=m: fc_mtile(qn, m))
                while order:
                    pop_filler()
            if _dbg:
                for t in range(4):
                    nc.sync.dma_start(out=ctxd[t], in_=ctx_t[t][:])
                    nc.sync.dma_start(out=qtd[t], in_=qt_t[t][:])
                    nc.sync.dma_start(out=ktd[t], in_=kt_t[t][:])

    nc.finalize()
    return nc


class _Runner:
    """Compile-once wrapper around the run_bass_via_pjrt shard_map path."""

    def __init__(self, nc):
        import jax
        from jax.sharding import Mesh, PartitionSpec

        from concourse import bass2jax, mybir as mb

        try:
            from jax.experimental.shard_map import shard_map
        except ImportError:
            from jax.shard_map import shard_map

        bass2jax.install_neuronx_cc_hook()
        self._nc = nc
        partition_name = (nc.partition_id_tensor.name
                          if nc.partition_id_tensor else None)
        in_names, out_names, out_avals = [], [], []
        self._zero_shapes = []
        for alloc in nc.m.functions[0].allocations:
            if not isinstance(alloc, mb.MemoryLocationSet):
                continue
            name = alloc.memorylocations[0].name
            if alloc.kind == "ExternalInput":
                if name != partition_name:
                    in_names.append(name)
            elif alloc.kind == "ExternalOutput":
                out_names.append(name)
                shape = tuple(alloc.tensor_shape)
                dtype = mb.dt.np(alloc.dtype)
                out_avals.append(jax.core.ShapedArray(shape, dtype))
                self._zero_shapes.append((shape, dtype))
        self._n_params = len(in_names)
        n_outs = len(out_avals)
        self._in_names = list(in_names)
        self._out_names = list(out_names)
        self._out_avals = out_avals
        all_in = in_names + out_names
        if partition_name is not None:
            all_in.append(partition_name)

        def _body(*args):
            operands = list(args)
            if partition_name is not None:
                operands.append(bass2jax.partition_id_tensor())
            return tuple(bass2jax._bass_exec_p.bind(
                *operands,
                out_avals=tuple(out_avals),
                in_names=tuple(all_in),
                out_names=tuple(out_names),
                lowering_input_output_aliases=(),
                sim_require_finite=True,
                sim_require_nnan=True,
                nc=nc,
            ))

        devices = jax.devices()[:NCORES]
        mesh = Mesh(np.asarray(devices), ("core",))
        self.mesh = mesh
        nin = self._n_params + n_outs
        self._sharded = jax.jit(
            shard_map(_body, mesh=mesh,
                      in_specs=(PartitionSpec("core"),) * nin,
                      out_specs=(PartitionSpec("core"),) * n_outs,
                      check_rep=False),
            donate_argnums=tuple(range(self._n_params, nin)),
            keep_unused=True,
        )

    def run(self, in_maps):
        import jax
        concat_in = [
            np.concatenate([np.asarray(in_maps[c][name])
                            for c in range(NCORES)], axis=0)
            for name in self._in_names
        ]
        concat_zeros = [np.zeros((NCORES * s[0], *s[1:]), d)
                        for s, d in self._zero_shapes]
        out_arrs = self._sharded(*concat_in, *concat_zeros)
        jax.block_until_ready(out_arrs)
        return [
            {name: np.asarray(out_arrs[i]).reshape(
                NCORES, *self._out_avals[i].shape)[c]
             for i, name in enumerate(self._out_names)}
            for c in range(NCORES)
        ]


_RUNNERS = {}


def _get_runner(lk, reps=1):
    key = (lk, reps)
    if key not in _RUNNERS:
        _RUNNERS[key] = _Runner(_build(lk, reps=reps))
    return _RUNNERS[key]


def _prep_in_maps(x, mask, Wq, bq, Wk, bk, Wv, bv, Wfc, bfc):
    """Shard + lay out the full inputs for the 8 cores.

    Returns (in_maps, lk) or (None, None) if the mask leaves more than LK
    keys unmasked in some batch (host fallback).
    """
    keep = [np.nonzero(mask[b] == 0)[0] for b in range(BS)]
    if max(len(kp) for kp in keep) > LK or min(len(kp) for kp in keep) == 0:
        # too many unmasked keys for the compiled shape, or a fully-masked
        # batch (reference degenerates to uniform attention there)
        return None, None
    lk = LK

    in_maps = []
    for c in range(NCORES):
        b, g = c // 2, c % 2
        sl = slice(g * HD, (g + 1) * HD)
        kp = keep[b]
        xkv_b = np.zeros((lk, D), np.float32)
        xkv_b[:len(kp)] = x[b][kp]
        biask = np.where(np.arange(lk) < len(kp), 0.0, NEGB).astype(np.float32)
        bias_cols = np.concatenate([
            bk[sl].reshape(4, 128).T,
            bq[sl].reshape(4, 128).T,
            (bfc * 0.5).reshape(8, 128).T,
        ], axis=1).astype(np.float32)
        in_maps.append({
            "xt": np.ascontiguousarray(
                x[b].T.reshape(8, 128, 4, 512).transpose(2, 1, 0, 3)
            ).astype(NPBF),
            "xkv": np.ascontiguousarray(
                xkv_b.T.reshape(8, 128, lk).transpose(1, 0, 2)).astype(NPBF),
            "wq": np.ascontiguousarray(
                Wq[:, sl].reshape(8, 128, 4, 128).transpose(2, 1, 0, 3)
            ).astype(NPBF),
            "wk": np.ascontiguousarray(
                Wk[:, sl].reshape(8, 128, 4, 128).transpose(2, 1, 0, 3)
            ).astype(NPBF),
            "wv": np.ascontiguousarray(
                Wv[:, sl].reshape(8, 128, HD).transpose(1, 0, 2)).astype(NPBF),
            "wfc": np.ascontiguousarray(
                Wfc[sl, :].reshape(4, 128, D).transpose(1, 0, 2)).astype(NPBF),
            "bias": np.ascontiguousarray(bias_cols),
            "bvr": np.ascontiguousarray(bv[sl]).reshape(1, HD).astype(NPBF),
            "mb": np.ascontiguousarray(biask.reshape(lk // 128, 128).T),
        })
    return in_maps, lk


def _host_reference(x, mask, Wq, bq, Wk, bk, Wv, bv, Wfc, bfc):
    """Numpy fallback, bit-compatible with the reference semantics."""
    out = np.empty((BS, L, D), np.float32)
    for b in range(BS):
        q = (x[b] @ Wq + bq).reshape(L, 16, DK).transpose(1, 0, 2)
        k = (x[b] @ Wk + bk).reshape(L, 16, DK).transpose(1, 0, 2)
        v = (x[b] @ Wv + bv).reshape(L, 16, DK).transpose(1, 0, 2)
        s = np.einsum("hqd,hkd->hqk", q, k) * SCALE
        m = mask[b].astype(np.float32)[None, None, :]
        s = s * (1.0 - m) + m * (-1e30)
        s = s - s.max(axis=-1, keepdims=True)
        p = np.exp(s)
        p /= p.sum(axis=-1, keepdims=True)
        o = np.einsum("hqk,hkd->hqd", p, v).transpose(1, 0, 2).reshape(L, D)
        out[b] = o @ Wfc + bfc
    return out


def kernel(x, mask, Wq, bq, Wk, bk, Wv, bv, Wfc, bfc, **_unused):
    x = np.asarray(x, np.float32)
    mask = np.asarray(mask)
    Wq, bq = np.asarray(Wq, np.float32), np.asarray(bq, np.float32)
    Wk, bk = np.asarray(Wk, np.float32), np.asarray(bk, np.float32)
    Wv, bv = np.asarray(Wv, np.float32), np.asarray(bv, np.float32)
    Wfc, bfc = np.asarray(Wfc, np.float32), np.asarray(bfc, np.float32)

    in_maps, lk = _prep_in_maps(x, mask, Wq, bq, Wk, bk, Wv, bv, Wfc, bfc)
    if in_maps is None:
        return _host_reference(x, mask, Wq, bq, Wk, bk, Wv, bv, Wfc, bfc)
    results = _get_runner(lk).run(in_maps)

    out = np.empty((BS, L, D), np.float32)
    for b in range(BS):
        p0 = results[2 * b]["out"].reshape(D, L).astype(np.float32)
        p1 = results[2 * b + 1]["out"].reshape(D, L).astype(np.float32)
        out[b] = (p0 + p1).T
    return out
